# revision 1
# baseline (speedup 1.0000x reference)
"""Trainium2 Bass kernel for nn_AttnBlock (per-pixel qk attention block).

Reference computation (per batch b):
  q = x @ wq.T ; k = x @ wk.T ; v = x @ wv.T          # [H*W, 512], heads n=8, d=64
  s[n, p]    = sum_d q[p, n*64+d] * k[p, n*64+d]      # per-pixel dot product
  w[n, h, :] = softmax(s[n, h, :] * d**-0.5)          # softmax over W axis (32)
  vsum[n, p] = sum_d v[p, n*64+d]
  out[b, n, hw, xy] = w[n, hw] * vsum[n, xy]          # outer product, 32 MB/batch

Sharding: data-parallel over batch: core b handles batch b (8 cores, B=8).
Output is write-bandwidth bound: 32 MB/core => ~90 us at ~358 GB/s.

Implementation notes (HW: ~131 us/looped-iter, ~126 us single-shot;
cost model 124 us; DMA roofline ~103 us):
- scores in fp16 (x, wq, wk cast on host): PE matmuls run 1 cyc/row vs 4 for
  fp32; output-relevant math (vsum, softmax, outer products) stays fp32.
  End-to-end rel err ~3e-4.
- x^T via PE transpose-mode (fp16 identity), 4 blocks packed per PSUM bank,
  one wide drain; vsum accumulates per chunk at high priority because
  vsum -> selector-matmul broadcast gates the first output DMA.
- softmax over W without max-subtraction (logits bounded ~N(0, 0.33)), one
  merged exp/reduce/reciprocal/divide per head-pair; weights transposed to
  [hw, n] with 8 PE transposes into one PSUM bank.
- outer products: DVE tensor_scalar (2x mode, 0.59us/tile) and ACT
  activation-Copy-with-scale (1x, 1.04us/tile) split 5:3.
- output: head-pair 8 MB DMAs on the SP HWDGE ring (HW-measured ~6 us
  faster than 8x4 MB: real DMAs amortize the ~2 us fixed receipt cost even
  though the cost model prices 8 MB at exactly 2x4 MB), double-buffered
  production tiles; the 4-DMA chain runs gapless.
"""

import numpy as np

import concourse.bass as bass
import concourse.mybir as mybir
import concourse.tile as tile
from concourse import bacc
from concourse.bass_utils import run_bass_kernel_spmd

F32 = mybir.dt.float32
F32R = mybir.dt.float32r
BF16 = mybir.dt.bfloat16
F16 = mybir.dt.float16

B, HW, DIM = 8, 1024, 512
N_HEADS, D_HEAD = 8, 64
N_CORES = 8
SCALE = float(D_HEAD) ** -0.5

# Use float32r (relaxed-precision fp32 matmul, 4x faster on PE) for the
# q/k projections, score reduction and vsum matmuls.
# NOTE: walrus requires fp32r operands to come from fp32r-rounding producers;
# no producer support exists in this stack, so keep False.
USE_FP32R = False

# Half-precision q/k score path: 4x faster PE matmuls than fp32; only touches
# the softmax logits. fp16 (not bf16): same PE rate, 3 more mantissa bits
# (score err ~5e-4 vs ~2e-3). Value ranges (|x|<~6, |w|<0.05, |q.k|<~30) are
# far inside fp16 limits. vsum + outer product stay fp32.
QK_BF16 = True
QK_HALF_DT = "float16"


def _mm_dt(ap):
    """View an fp32 AP as float32r for faster PE matmuls when enabled."""
    if USE_FP32R:
        return ap.bitcast(F32R)
    return ap


def build_program(loop_iters=None):
    """loop_iters: if set, wrap the whole kernel body in a tc.For_i hardware
    loop (benchmarking only — one NEFF executes the body N times)."""
    # Bacc (not raw Bass): its compile() runs move_matmul_waits_to_ldweights,
    # without which any matmul with >1 semaphore wait fails walrus codegen.
    nc = bacc.Bacc(None)

    qk_dt = (F16 if QK_HALF_DT == "float16" else BF16) if QK_BF16 else F32

    x_d = nc.declare_dram_parameter("x", [HW, DIM], qk_dt, isOutput=False)
    wqt_d = nc.declare_dram_parameter("wqt", [DIM, DIM], qk_dt, isOutput=False)
    wkt_d = nc.declare_dram_parameter("wkt", [DIM, DIM], qk_dt, isOutput=False)
    wvt_d = nc.declare_dram_parameter("wvt", [DIM, N_HEADS], qk_dt, isOutput=False)
    ind2_d = nc.declare_dram_parameter("ind2", [128, 2], qk_dt, isOutput=False)
    ident_d = nc.declare_dram_parameter("ident", [128, 128], qk_dt, isOutput=False)
    ident2_d = nc.declare_dram_parameter("ident2", [2, 2], F32, isOutput=False)
    sel_d = nc.declare_dram_parameter("sel", [N_HEADS, N_HEADS * 128], F32,
                                      isOutput=False)
    y_d = nc.declare_dram_parameter("y", [N_HEADS, HW, HW], F32, isOutput=True)

    with tile.TileContext(nc) as tc:
        with (
            tc.tile_pool(name="singles", bufs=1) as singles,
            tc.tile_pool(name="qt", bufs=2) as qtp,
            tc.tile_pool(name="kt", bufs=2) as ktp,
            tc.tile_pool(name="sprod", bufs=2) as sprodp,
            tc.tile_pool(name="smax", bufs=2) as smaxp,
            tc.tile_pool(name="wt", bufs=2) as wtp,
            tc.tile_pool(name="bc", bufs=2) as bcp,
            tc.tile_pool(name="prod", bufs=2) as prodp,
            tc.tile_pool(name="tp_ps", bufs=2, space="PSUM") as tp_ps,
            tc.tile_pool(name="qk_ps", bufs=2, space="PSUM") as qk_ps,
            tc.tile_pool(name="s_ps", bufs=1, space="PSUM") as s_ps,
            tc.tile_pool(name="v_ps", bufs=1, space="PSUM") as v_ps,
        ):
            def emit_body():
                # ---- constant / weight loads -------------------------------
                # x arrives fp16 (halves load bytes); transposed on the PE
                # (fp16 transpose-mode is 1 cyc/row) with a single fp16 drain
                # per chunk. The vsum -> broadcast chain gates the first
                # output DMA, so it accumulates per-chunk at high priority.
                ident_sb = singles.tile([128, 128], qk_dt)
                nc.gpsimd.dma_start(out=ident_sb, in_=ident_d[:])
                ident2_sb = singles.tile([2, 2], F32)
                nc.gpsimd.dma_start(out=ident2_sb, in_=ident2_d[:])
                wvt_sb = singles.tile([128, 4, N_HEADS], qk_dt)
                nc.gpsimd.dma_start(
                    out=wvt_sb, in_=wvt_d[:].rearrange("(k p) n -> p k n", p=128)
                )
                x_sb = singles.tile([128, 8, DIM], qk_dt)
                xv = x_d[:].rearrange("(g p) c -> p g c", p=128)
                nc.sync.dma_start(out=x_sb[:, 0:4, :], in_=xv[:, 0:4, :])
                nc.sync.dma_start(out=x_sb[:, 4:8, :], in_=xv[:, 4:8, :])
                wqt_sb = singles.tile([128, 4, DIM], qk_dt)
                nc.gpsimd.dma_start(
                    out=wqt_sb, in_=wqt_d[:].rearrange("(k p) o -> p k o", p=128)
                )
                wkt_sb = singles.tile([128, 4, DIM], qk_dt)
                nc.gpsimd.dma_start(
                    out=wkt_sb, in_=wkt_d[:].rearrange("(k p) o -> p k o", p=128)
                )
                ind2_sb = singles.tile([128, 2], qk_dt)
                nc.gpsimd.dma_start(out=ind2_sb, in_=ind2_d[:])
                sel_sb = singles.tile([N_HEADS, N_HEADS * 128], F32)
                nc.gpsimd.dma_start(out=sel_sb, in_=sel_d[:])

                # ---- x^T (PE transpose) + per-chunk vsum -------------------
                xTb = singles.tile([128, 4, HW], qk_dt, name="xTb")
                vps = v_ps.tile([N_HEADS, HW], F32)
                vsum_sb = singles.tile([N_HEADS, HW], F32)
                for hj in range(8):
                    tp = tp_ps.tile([128, 512], qk_dt, tag="tp")
                    for ci in range(4):
                        nc.tensor.transpose(
                            tp[:, ci * 128 : (ci + 1) * 128],
                            x_sb[:, hj, ci * 128 : (ci + 1) * 128],
                            ident_sb,
                        )
                    hsl = slice(hj * 128, (hj + 1) * 128)
                    if hj % 2 == 0:
                        nc.vector.tensor_copy(
                            xTb[:, :, hsl], tp.rearrange("p (c m) -> p c m", c=4)
                        )
                    else:
                        nc.scalar.copy(
                            xTb[:, :, hsl], tp.rearrange("p (c m) -> p c m", c=4)
                        )
                    # vsum[n, xy in chunk hj] += wv_sum^T . x^T chunk
                    with tc.high_priority():
                        for ki in range(4):
                            nc.tensor.matmul(
                                vps[:, hsl],
                                wvt_sb[:, ki, :],
                                xTb[:, ki, hsl],
                                start=(ki == 0),
                                stop=(ki == 3),
                            )
                with tc.high_priority():
                    nc.scalar.copy(vsum_sb, vps)

                # ---- per head-pair pipeline --------------------------------
                def emit_qk(ti):
                    """Q^T/K^T chunk ti (heads 2ti, 2ti+1), then scores+softmax.

                    Returns per-half softmax weight tiles (in [n2, hw] layout)."""
                    qt_sb = qtp.tile([128, HW], qk_dt, tag="qt")
                    kt_sb = ktp.tile([128, HW], qk_dt, tag="kt")
                    sprod = sprodp.tile([128, HW], qk_dt, tag="sp")
                    sps = s_ps.tile([2, HW], F32, tag="s")
                    w_halves = []
                    for nj in range(2):
                        for dst_sb, w_sb, eng in (
                            (qt_sb, wqt_sb, "dve"),
                            (kt_sb, wkt_sb, "act"),
                        ):
                            ps = qk_ps.tile([128, 512], F32, tag="qk")
                            for ki in range(4):
                                nc.tensor.matmul(
                                    ps,
                                    w_sb[:, ki, ti * 128 : (ti + 1) * 128],
                                    xTb[:, ki, nj * 512 : (nj + 1) * 512],
                                    start=(ki == 0),
                                    stop=(ki == 3),
                                )
                            dst = dst_sb[:, nj * 512 : (nj + 1) * 512]
                            if eng == "dve":
                                # fp16-out PSUM drain is 0.39us on DVE vs
                                # 0.61us on ACT; split to shorten the chain.
                                nc.vector.tensor_copy(dst, ps)
                            else:
                                nc.scalar.copy(dst, ps)

                        sl = slice(nj * 512, (nj + 1) * 512)
                        nc.vector.tensor_tensor(
                            out=sprod[:, sl], in0=qt_sb[:, sl], in1=kt_sb[:, sl],
                            op=mybir.AluOpType.mult,
                        )
                        nc.tensor.matmul(
                            sps[:, sl], ind2_sb, sprod[:, sl],
                            start=True, stop=True,
                        )
                        # per-half softmax, pipelined: exp(nj=0) on ACT runs
                        # while nj=1 scores are still in flight, and the DVE
                        # reduce/divide of nj=0 overlaps exp(nj=1). No
                        # max-subtraction needed: logits ~N(0, 0.33).
                        e_h = smaxp.tile([2, 512], F32, tag="e")
                        nc.scalar.activation(
                            out=e_h, in_=sps[:, sl],
                            func=mybir.ActivationFunctionType.Exp,
                            scale=SCALE,
                        )
                        denom = smaxp.tile([2, 16], F32, tag="d")
                        nc.vector.tensor_reduce(
                            out=denom,
                            in_=e_h.rearrange("p (h w) -> p h w", w=32),
                            axis=mybir.AxisListType.X,
                            op=mybir.AluOpType.add,
                        )
                        rden = smaxp.tile([2, 16], F32, tag="r")
                        nc.vector.reciprocal(rden, denom)
                        w_h = smaxp.tile([2, 512], F32, tag="w")
                        rden_b = bass.AP(
                            tensor=rden.tensor,
                            offset=rden.offset,
                            ap=[*rden.ap, [0, 32]],
                        )
                        nc.vector.tensor_tensor(
                            out=w_h.rearrange("p (h w) -> p h w", w=32),
                            in0=e_h.rearrange("p (h w) -> p h w", w=32),
                            in1=rden_b,
                            op=mybir.AluOpType.mult,
                        )
                        w_halves.append(w_h)
                    return w_halves

                def emit_wt(w_halves):
                    """Transpose softmax weights to [hw, n2] layout: 8 PE
                    transposes into one PSUM bank, one DVE drain."""
                    wt_sb = wtp.tile([128, 8, 2], F32, tag="wt")
                    tp = tp_ps.tile([128, 16], F32, tag="tp")
                    for cj in range(8):
                        nc.tensor.transpose(
                            tp[:, cj * 2 : (cj + 1) * 2],
                            w_halves[cj // 4][:, (cj % 4) * 128 : (cj % 4 + 1) * 128],
                            ident2_sb,
                        )
                    nc.vector.tensor_copy(
                        wt_sb, tp.rearrange("p (c n) -> p c n", c=8)
                    )
                    return wt_sb

                def emit_production(ti, wt_sb):
                    """Broadcast vsum rows (PE selector outer product), then the
                    outer products w x vsum for heads 2ti, 2ti+1, then DMA out."""
                    bcast_t = bcp.tile([128, 2, HW], F32, tag="bc")
                    for j in range(2):
                        head = 2 * ti + j
                        for nj in range(2):
                            bps = qk_ps.tile([128, 512], F32, tag="qk")
                            nc.tensor.matmul(
                                bps,
                                sel_sb[:, head * 128 : (head + 1) * 128],
                                vsum_sb[:, nj * 512 : (nj + 1) * 512],
                                start=True,
                                stop=True,
                            )
                            dstb = bcast_t[:, j, nj * 512 : (nj + 1) * 512]
                            # both on DVE: keeps ACT free for exp, which is
                            # on the first-output-DMA critical path
                            nc.vector.tensor_copy(dstb, bps)
                    # both heads of one pair go into a single tile and a
                    # single 8 MB DMA. The cost model prices 8 MB at exactly
                    # 2x 4 MB (no amortization), but HW-measured DMA rates
                    # (1MB->341, 16MB->425 GB/s) show real transfers amortize
                    # the ~2us fixed receipt cost: ~400 vs ~350 GB/s here.
                    prod_t = prodp.tile([128, 2, 8, HW], F32, tag="pr")
                    for j in range(2):
                        for cj in range(8):
                            dst = prod_t[:, j, cj, :]
                            src = bcast_t[:, j, :]
                            sc = wt_sb[:, cj, j : j + 1]
                            # 5:3 DVE:ACT split -- DVE fp32 tensor_scalar runs
                            # 2x (0.59us), ACT fp32 activation only 1x (1.04us)
                            if cj % 8 in (0, 2, 4, 6, 7):
                                nc.vector.tensor_scalar_mul(dst, src, sc)
                            else:
                                nc.scalar.activation(
                                    out=dst, in_=src,
                                    func=mybir.ActivationFunctionType.Copy, scale=sc,
                                )
                    nc.sync.dma_start(
                        out=y_d[2 * ti : 2 * ti + 2].rearrange(
                            "n (c p) xy -> p n c xy", p=128
                        ),
                        in_=prod_t,
                    )

                # strict dependency-order emission: production (which feeds the
                # output DMA, the kernel's bottleneck) gets the earliest priority
                # on DVE/ACT. PE bubbles while waiting on softmax are irrelevant
                # (PE has ~4x slack).
                for ti in range(4):
                    w_halves = emit_qk(ti)
                    emit_production(ti, emit_wt(w_halves))


            if loop_iters:
                with tc.For_i(0, loop_iters, 1):
                    emit_body()
            else:
                emit_body()

    nc.compile()
    return nc


_NC_CACHE = None


def _get_nc():
    global _NC_CACHE
    if _NC_CACHE is None:
        _NC_CACHE = build_program()
    return _NC_CACHE


def make_in_maps(x, wq, wk, wv):
    """Host-side input prep: weight-layout transforms (cheap, O(dim^2)) and
    per-core batch sharding."""
    import ml_dtypes

    x = np.ascontiguousarray(np.asarray(x, dtype=np.float32))
    wq = np.asarray(wq, dtype=np.float32)
    wk = np.asarray(wk, dtype=np.float32)
    wv = np.asarray(wv, dtype=np.float32)
    b, H, W, dim = x.shape
    assert (b, H, W, dim) == (B, 32, 32, DIM)

    half = np.float16 if QK_HALF_DT == "float16" else ml_dtypes.bfloat16
    qk_np = half if QK_BF16 else np.float32
    wqt = np.ascontiguousarray(wq.T).astype(qk_np)         # [c, o]
    wkt = np.ascontiguousarray(wk.T).astype(qk_np)         # [c, o]
    wvt = np.ascontiguousarray(
        wv.reshape(N_HEADS, D_HEAD, DIM).sum(axis=1).T     # [c, n]
    ).astype(qk_np)
    ind2 = np.zeros((128, 2), dtype=np.float32)
    ind2[np.arange(128), np.arange(128) // D_HEAD] = 1.0
    ind2 = ind2.astype(qk_np)
    ident = np.eye(128, dtype=np.float32)
    sel = np.zeros((N_HEADS, N_HEADS * 128), dtype=np.float32)
    for n in range(N_HEADS):
        sel[n, n * 128 : (n + 1) * 128] = 1.0

    return [
        {
            "x": np.ascontiguousarray(x[i].reshape(HW, DIM)).astype(qk_np),
            "wqt": wqt,
            "wkt": wkt,
            "wvt": wvt,
            "ind2": ind2,
            "ident": ident.astype(qk_np),
            "ident2": np.eye(2, dtype=np.float32),
            "sel": sel,
        }
        for i in range(N_CORES)
    ]


def kernel(x, wq, wk, wv):
    nc = _get_nc()
    in_maps = make_in_maps(x, wq, wk, wv)
    res = run_bass_kernel_spmd(nc, in_maps, list(range(N_CORES)))
    out = np.stack([res.results[i]["y"] for i in range(N_CORES)], axis=0)
    # [b, n, hw, xy] -> [b, n, h, w, x, y]
    return out.reshape(B, N_HEADS, 32, 32, 32, 32)


if __name__ == "__main__":
    rng = np.random.default_rng(0)
    x = rng.standard_normal((B, 32, 32, DIM), dtype=np.float32)
    s = 1.0 / np.sqrt(DIM)
    wq = rng.uniform(-s, s, (DIM, DIM)).astype(np.float32)
    wk = rng.uniform(-s, s, (DIM, DIM)).astype(np.float32)
    wv = rng.uniform(-s, s, (DIM, DIM)).astype(np.float32)
    y = kernel(x=x, wq=wq, wk=wk, wv=wv)
    print(y.shape, y.dtype)

